# revision 4
# baseline (speedup 1.0000x reference)
"""CTRNN cell (6 Euler unfolds) on 8 Trainium2 NeuronCores.

Math (per unfold, 6x):
    f     = tanh([x, s] @ W + b)
    s_new = s + 0.1 * (-s + f)  = 0.9*s + 0.1*f

Strategy (v2):
  - Data-parallel over batch: B=8192 -> 1024 rows/core, no cross-core
    communication. Host does the cheap numpy transposes/packing.
  - Everything transposed on-chip (features on SBUF partitions, batch on
    the free dim) so W slices are directly the stationary lhsT and the
    batch is the moving free dim.
  - ALL matmul operands are fp16 (verified 1.6e-3 rel err vs the 2e-2
    gate): fp16 streams at 1 col/cycle like bf16 (f32 runs the slow
    FP32-HIGH path), halves every DMA and SBUF read, and unlocks the
    DVE 2x 16-bit mode for the elementwise chain.
  - Delta form: psum holds z_k = [x,s0]@W' + sum 0.1*tmp_i@Wb across all
    unfolds (one PSUM (128,1024) span per m-tile = all 8 banks, never
    restarted).  Host pre-scales: s10 = 10*s0, wb = 0.1*Wb, so
    z_0 = x@Wt + s10@wb and z_{k+1} = z_k + tmp_k@wb.
  - State is never materialized.  tmp_k = f_k - s_k obeys
        tmp_{k+1} = f_{k+1} + u_k,   u_k = 0.9*tmp_k - f_k,
    so the only critical-path op between tanh and the next matmul round
    is ONE tensor_tensor add; the u pass runs in scheduling slack.
    Output: s_6 = f_5 - 0.9*tmp_5 (one fused scalar_tensor_tensor).
  - Two batch-chunk streams (512 cols each) are interleaved: while the
    tensor engine runs chunk A's 16-matmul round, chunk B's tanh->tmp
    chain completes, so unfold boundaries cost no PE idle time.
  - DMA: Wt+x on the sync (SP) HWDGE ring (fastest starter), s10 on the
    scalar (ACT) ring, wb+bias on SWDGE; all host-packed (128, k*C)
    so per-partition runs are 4KB descriptors.  Output fp16 per
    (chunk, m-tile) tile on sync/gpsimd as soon as it is ready.
  - Junk warm-up matmuls (N=256, fine-grained) span the load phase so
    the HAM clock gate is at 8/8 when the first real matmul issues.
"""

import numpy as np

UNFOLDS = 6
B, D, N = 8192, 512, 512
NCORES = 8
BC = B // NCORES          # batch rows per core
CH = 512                  # chunk = matmul moving free dim (PSUM bank)
NCH = BC // CH            # 2
P = 128
KT = D // P               # 4 k-tiles for x (and for s / tmp)
MT = N // P               # 4 m-tiles of the output dim
NJUNK = 22                # warm-up matmuls (N=256) spanning the load

_compiled_nc = None


def _build_nc():
    import concourse.bass as bass  # noqa: F401
    import concourse.bacc as bacc
    import concourse.tile as tile
    from concourse import mybir

    f32 = mybir.dt.float32
    f16 = mybir.dt.float16
    MULT = mybir.AluOpType.mult
    ADD = mybir.AluOpType.add
    SUB = mybir.AluOpType.subtract
    TANH = mybir.ActivationFunctionType.Tanh

    nc = bacc.Bacc("TRN2", target_bir_lowering=False, debug=False)

    xP = nc.dram_tensor("xP", [P, KT * BC], f16, kind="ExternalInput").ap()
    sP = nc.dram_tensor("sP", [P, KT * BC], f16, kind="ExternalInput").ap()
    wtP = nc.dram_tensor("wtP", [P, KT * N], f16, kind="ExternalInput").ap()
    wbP = nc.dram_tensor("wbP", [P, KT * N], f16, kind="ExternalInput").ap()
    bias = nc.dram_tensor("bias", [N], f32, kind="ExternalInput").ap()
    outT = nc.dram_tensor("outT", [N, BC], f16, kind="ExternalOutput").ap()

    with tile.TileContext(nc) as tc:
        with (
            tc.tile_pool(name="weights", bufs=1) as wpool,
            tc.tile_pool(name="data", bufs=1) as data,
            tc.tile_pool(name="fpool", bufs=2) as fpool,
            tc.tile_pool(name="tpool", bufs=2) as tpool,
            tc.tile_pool(name="upool", bufs=2) as upool,
            tc.tile_pool(name="opool", bufs=1) as opool,
            tc.tile_pool(name="psum", bufs=1, space="PSUM") as psump,
        ):
            # ---- warm-up junk + input DMAs --------------------------------
            junk = wpool.tile([P, 256], f16, tag="junk", name="junk")
            nc.gpsimd.memset(junk[:], 0)

            # sync ring: Wt then x (first-matmul gate, fastest ring)
            wt_mega = wpool.tile([P, KT * N], f16, tag="wt", name="wt_mega")
            nc.sync.dma_start(wt_mega[:], wtP[:, :])
            HALF = KT * BC // 2
            x_mega = data.tile([P, KT * BC], f16, tag="xm", name="x_mega")
            nc.sync.dma_start(x_mega[:, 0:HALF], xP[:, 0:HALF])
            nc.sync.dma_start(x_mega[:, HALF:], xP[:, HALF:])

            # scalar ring: s10 halves (needed mid-round-0)
            s_mega = data.tile([P, KT * BC], f16, tag="sm", name="s_mega")
            nc.scalar.dma_start(s_mega[:, 0:HALF], sP[:, 0:HALF])
            nc.scalar.dma_start(s_mega[:, HALF:], sP[:, HALF:])

            # SWDGE: wb (needed at round 1) + bias (needed at first tanh)
            wb_mega = wpool.tile([P, KT * N], f16, tag="wb", name="wb_mega")
            nc.gpsimd.dma_start(wb_mega[:], wbP[:, :])
            bias_sb = wpool.tile([P, MT], f32, tag="bias", name="bias_sb")
            nc.gpsimd.dma_start(bias_sb[:], bias.rearrange("(m p) -> p m", p=P))

            wt = [wt_mega[:, j * N:(j + 1) * N] for j in range(KT)]
            wb = [wb_mega[:, j * N:(j + 1) * N] for j in range(KT)]
            x_sb = [x_mega[:, j * BC:(j + 1) * BC] for j in range(KT)]
            s_sb = [s_mega[:, j * BC:(j + 1) * BC] for j in range(KT)]

            # ---- persistent PSUM accumulators (all 8 banks) ---------------
            ps = [psump.tile([P, BC], f32, tag=f"ps{m}", name=f"ps{m}")
                  for m in range(MT)]

            # HAM warm-up: junk matmuls keep the PE busy while inputs
            # stream in so the first real matmul runs at 2.4 GHz.
            for r in range(NJUNK):
                nc.tensor.matmul(
                    ps[r % MT][:, 0:256],
                    lhsT=junk[:, 0:P], rhs=junk[:, 0:256],
                    start=True, stop=True, skip_group_check=True,
                )

            def mm_round(c, weights, rhs_slices, first=False, last=False):
                # one chunk's matmul round: psum[m][:,chunk] += sum_j ...
                cs = c * CH
                nkt = len(weights)
                for j in range(nkt):
                    for m in range(MT):
                        nc.tensor.matmul(
                            ps[m][:, cs:cs + CH],
                            lhsT=weights[j][:, m * P:(m + 1) * P],
                            rhs=rhs_slices[j],
                            start=(first and j == 0),
                            stop=(last and j == nkt - 1),
                            skip_group_check=True,
                        )

            # round 0: z = x@Wt + s10@wb  (per chunk)
            for c in range(NCH):
                cs = c * CH
                mm_round(c, wt + wb,
                         [t[:, cs:cs + CH] for t in (x_sb + s_sb)],
                         first=True)

            # ---- unfolds: interleaved chunk streams -----------------------
            f_t = [[None] * MT for _ in range(NCH)]
            tmp_t = [[None] * MT for _ in range(NCH)]
            u_t = [[None] * MT for _ in range(NCH)]
            for k in range(UNFOLDS):
                last = k == UNFOLDS - 1
                for c in range(NCH):
                    cs = c * CH
                    for m in range(MT):
                        f = fpool.tile([P, CH], f16, tag=f"f{c}_{m}",
                                       name=f"f{k}_{c}_{m}")
                        nc.scalar.activation(
                            f[:], ps[m][:, cs:cs + CH], TANH,
                            bias=bias_sb[:, m:m + 1], scale=1.0,
                        )
                        t = tpool.tile([P, CH], f16, tag=f"t{c}_{m}",
                                       name=f"t{k}_{c}_{m}")
                        if k == 0:
                            # tmp0 = f0 - s0 = (s10 * -0.1) + f0
                            nc.vector.scalar_tensor_tensor(
                                t[:], s_sb[m][:, cs:cs + CH], -0.1, f[:],
                                op0=MULT, op1=ADD,
                            )
                        else:
                            # tmp_k = f_k + u_{k-1}
                            nc.vector.tensor_tensor(
                                t[:], f[:], u_t[c][m][:], ADD,
                            )
                        f_t[c][m], tmp_t[c][m] = f, t
                        if not last:
                            # u_k = 0.9*tmp_k - f_k   (off critical path)
                            u = upool.tile([P, CH], f16, tag=f"u{c}_{m}",
                                           name=f"u{k}_{c}_{m}")
                            nc.vector.scalar_tensor_tensor(
                                u[:], t[:], 0.9, f[:], op0=MULT, op1=SUB,
                            )
                            u_t[c][m] = u
                        else:
                            # s6 = f5 - 0.9*tmp5 ; DMA out immediately
                            o = opool.tile([P, CH], f16, tag=f"o{c}_{m}",
                                           name=f"o{c}_{m}")
                            nc.vector.scalar_tensor_tensor(
                                o[:], t[:], -0.9, f[:], op0=MULT, op1=ADD,
                            )
                            eng = nc.sync if (m % 2 == 0) else nc.gpsimd
                            eng.dma_start(
                                outT[m * P:(m + 1) * P, cs:cs + CH], o[:])
                    if not last:
                        # next round's matmuls for this chunk
                        mm_round(c, wb, [t[:] for t in tmp_t[c]],
                                 last=(k == UNFOLDS - 2))

    nc.compile()
    return nc


def _get_nc():
    global _compiled_nc
    if _compiled_nc is None:
        _compiled_nc = _build_nc()
    return _compiled_nc


def make_in_maps(x, s, W, b):
    """Shard + pack host-side: everything fp16, (128, k*C) layouts with
    k-tiles side by side so per-partition DMA runs are 4KB contiguous."""
    xT = np.ascontiguousarray(x.T)            # (D, B) f32
    sT = np.ascontiguousarray(10.0 * s.T)     # (N, B) f32, pre-scaled
    wt = np.ascontiguousarray(
        W[:D].reshape(KT, P, N).transpose(1, 0, 2).reshape(P, -1)
    ).astype(np.float16)
    wb = np.ascontiguousarray(
        (0.1 * W[D:]).reshape(KT, P, N).transpose(1, 0, 2).reshape(P, -1)
    ).astype(np.float16)
    in_maps = []
    for c in range(NCORES):
        sl = slice(c * BC, (c + 1) * BC)
        xs = xT[:, sl].reshape(KT, P, BC).transpose(1, 0, 2).reshape(P, -1)
        ss = sT[:, sl].reshape(KT, P, BC).transpose(1, 0, 2).reshape(P, -1)
        in_maps.append({
            "xP": np.ascontiguousarray(xs).astype(np.float16),
            "sP": np.ascontiguousarray(ss).astype(np.float16),
            "wtP": wt,
            "wbP": wb,
            "bias": np.ascontiguousarray(b.astype(np.float32)),
        })
    return in_maps


def kernel(**inputs):
    from concourse.bass_utils import run_bass_kernel_spmd

    x = np.asarray(inputs["inputs"], dtype=np.float32)
    s = np.asarray(inputs["state"], dtype=np.float32)
    W = np.ascontiguousarray(np.asarray(inputs["W"], dtype=np.float32))
    b = np.ascontiguousarray(np.asarray(inputs["bias"], dtype=np.float32))

    in_maps = make_in_maps(x, s, W, b)
    nc = _get_nc()
    res = run_bass_kernel_spmd(nc, in_maps, list(range(NCORES))).results
    outT = np.concatenate([res[c]["outT"] for c in range(NCORES)], axis=1)
    out = np.ascontiguousarray(outT.T).astype(np.float32)
    return (out, out)
